# revision 1
# baseline (speedup 1.0000x reference)
"""Trainium2 Bass kernel for nn_InnerProductDecoder.

For each graph b: out[b] = P_b @ P_b^T where P_b is the zero-padded
[max_n, D] node-feature matrix of graph b (pad_sequence equivalent).

Strategy: data parallel over B (64 graphs / 8 cores). Graphs are
sorted by length and dealt round-robin so slot s on every core holds
a graph of length <= slot_len[s]; the SPMD program is built for the
slot-length profile, so each core only moves/computes its graphs'
ragged extents (zero rows/cols of the output are never written on
device — the host pastes valid [n_b, n_b] blocks into a zeros array).

Per-core raw-Bass 4-engine pipeline (Tile's drain doesn't compile on
this walrus build):

  sync  (SP) : input DMA  xt slot block      -> SBUF x-buffer (x3)
  tensor(PE) : 4*ceil(L/128) f32r matmuls    -> <=4 PSUM banks (x2 sets)
  vector(DVE): PSUM -> SBUF out-buffer copies (x3)
  scalar(ACT): output DMA out-buffer         -> out slot block

All sems are cleared at the tail (barrier + sem_clear) so re-executing
the loaded NEFF stays correct.
"""

import numpy as np

N_CORES = 8
B = 64
MAXN = 512
D = 512
PER_CORE = B // N_CORES  # 8 slots per core
KCH = D // 128  # 4 contraction chunks
USE_F32R = True

_prog_cache = {}


def _mb(l):
    return (l + 127) // 128


def _build_program(slot_lens):
    import concourse.bass as bass
    from concourse import mybir

    f32 = mybir.dt.float32
    nc = bass.Bass()

    J = len(slot_lens)
    lmax = max(slot_lens)
    in_off = np.concatenate([[0], np.cumsum([KCH * l for l in slot_lens])])
    out_off = np.concatenate([[0], np.cumsum([_mb(l) * l for l in slot_lens])])
    # m-blocks completed after slot j (prefix sums for sem values)
    cum_mb = np.concatenate([[0], np.cumsum([_mb(l) for l in slot_lens])])

    xt = nc.dram_tensor("xt", [128, int(in_off[-1])], f32, kind="ExternalInput")
    out = nc.dram_tensor("out", [128, int(out_off[-1])], f32, kind="ExternalOutput")

    # Output DMA plan per slot: write full-height blocks in one DMA and the
    # partial last m-block with only its valid rows (avoids reading
    # uninitialized SBUF rows and skips dead output bytes).
    # Each entry: list of (rows, col_lo, col_hi) into the slot's ob region.
    out_plan = []
    for L in slot_lens:
        mb, rl = _mb(L), L - 128 * (_mb(L) - 1)
        if rl == 128:
            out_plan.append([(128, 0, mb * L)])
        elif mb == 1:
            out_plan.append([(rl, 0, L)])
        else:
            out_plan.append([(128, 0, (mb - 1) * L), (rl, (mb - 1) * L, mb * L)])
    ndma = [len(p) for p in out_plan]
    # cumulative out-DMA count per buffer index i after each slot
    cum_out = {i: [0] for i in range(3)}
    for j in range(J):
        for i in range(3):
            cum_out[i].append(cum_out[i][-1] + (ndma[j] if j % 3 == i else 0))

    from contextlib import ExitStack

    with ExitStack() as st:
        xdt = mybir.dt.float32r if USE_F32R else f32
        xb = [
            st.enter_context(nc.sbuf_tensor(f"xb{i}", [128, KCH * lmax], xdt))
            for i in range(3)
        ]
        ob = [
            st.enter_context(nc.sbuf_tensor(f"ob{i}", [128, 4 * lmax], f32))
            for i in range(3)
        ]
        ps = [
            st.enter_context(nc.psum_tensor(f"ps{i}", [128, 512], f32))
            for i in range(8)
        ]
        in_sems = [st.enter_context(nc.semaphore(f"in_sem{i}")) for i in range(3)]
        out_sems = [st.enter_context(nc.semaphore(f"out_sem{i}")) for i in range(3)]
        mm_sem = st.enter_context(nc.semaphore("mm_sem"))
        cp_sem = st.enter_context(nc.semaphore("cp_sem"))

        blk_ctx = nc.Block()
        block = blk_ctx.__enter__()

        @block.sync
        def _(sync):
            for j in range(J):
                if j >= 3:
                    # x-buffer j%3 free once slot j-3's last matmul ran
                    sync.wait_ge(mm_sem, int(cum_mb[j - 2]))
                w = KCH * slot_lens[j]
                src = xt[:, int(in_off[j]) : int(in_off[j]) + w]
                if USE_F32R:
                    src = src.bitcast(xdt)
                sync.dma_start(xb[j % 3][:, :w], src).then_inc(in_sems[j % 3], 16)
            for i in range(3):
                sync.wait_ge(out_sems[i], 16 * cum_out[i][J])

        @block.tensor
        def _(tensor):
            for j in range(J):
                L = slot_lens[j]
                tensor.wait_ge(in_sems[j % 3], 16 * (j // 3 + 1))
                if j >= 2:
                    # PSUM bank set j%2 free once slot j-2 fully copied out
                    tensor.wait_ge(cp_sem, int(cum_mb[j - 1]))
                pb = (j % 2) * 4
                for m in range(_mb(L)):
                    rows = min(128, L - 128 * m)
                    for k in range(KCH):
                        lhsT = xb[j % 3][:, k * L + m * 128 : k * L + m * 128 + rows]
                        rhs = xb[j % 3][:, k * L : (k + 1) * L]
                        ins = nc.tensor.matmul(
                            ps[pb + m][:rows, :L], lhsT, rhs,
                            start=(k == 0), stop=(k == KCH - 1),
                        )
                        if k == KCH - 1:
                            ins.then_inc(mm_sem, 1)

        @block.vector
        def _(vector):
            for j in range(J):
                L = slot_lens[j]
                if j >= 3:
                    # out-buffer j%3 free once slot j-3's output DMAs landed
                    vector.wait_ge(out_sems[j % 3], 16 * cum_out[j % 3][j - 2])
                pb = (j % 2) * 4
                for m in range(_mb(L)):
                    rows = min(128, L - 128 * m)
                    vector.wait_ge(mm_sem, int(cum_mb[j]) + m + 1)
                    nc.vector.tensor_copy(
                        ob[j % 3][:rows, m * L : (m + 1) * L], ps[pb + m][:rows, :L]
                    ).then_inc(cp_sem, 1)

        @block.scalar
        def _(scalar):
            for j in range(J):
                scalar.wait_ge(cp_sem, int(cum_mb[j + 1]))
                o0 = int(out_off[j])
                for rows, lo, hi in out_plan[j]:
                    scalar.dma_start(
                        out[:rows, o0 + lo : o0 + hi], ob[j % 3][:rows, lo:hi]
                    ).then_inc(out_sems[j % 3], 16)

        blk_ctx.__exit__(None, None, None)

        # Reset all sems to 0 so re-executing the loaded NEFF stays correct
        # (NRT does not clear sems between executes). Same sequence Tile uses.
        all_sems = in_sems + out_sems + [mm_sem, cp_sem]
        nc.all_engine_barrier()
        sem_nums = sorted(s.num for s in all_sems)
        lo, hi = sem_nums[0], sem_nums[-1] + 1
        assert sem_nums == list(range(lo, hi)), sem_nums
        nc.gpsimd.dma_reset(range(lo, hi))
        nc.gpsimd.sem_clear(range(lo, hi))
        nc.all_engine_barrier()

    return nc


def _plan(graph_ids):
    """Sort graphs by length desc, deal round-robin: slot s of core c gets
    rank 8*s + c. slot_lens[s] = max length within the slot = rank 8*s."""
    lengths = np.bincount(graph_ids, minlength=B)
    order = np.argsort(-lengths, kind="stable")  # rank -> graph id
    # round up to multiple of 16 so every k*L*4-byte SBUF operand offset
    # stays 64B-aligned (walrus ISA check on matmul operands)
    slot_lens = tuple(
        (int(lengths[order[N_CORES * s]]) + 15) // 16 * 16 for s in range(PER_CORE)
    )
    assign = order.reshape(PER_CORE, N_CORES)  # [slot, core] -> graph id
    return lengths, assign, slot_lens


def _get_program(slot_lens):
    if slot_lens not in _prog_cache:
        _prog_cache[slot_lens] = _build_program(list(slot_lens))
    return _prog_cache[slot_lens]


def _host_prepare(batched_h, graph_ids, pos_ids, lengths, assign, slot_lens):
    """Build per-core [128, sum(KCH*L_s)] inputs:
    xt[p, in_off[s] + k*L_s + n] = h[g_cs][node n][d=128k+p]."""
    padded = np.zeros((B, MAXN, D), dtype=np.float32)
    padded[graph_ids, pos_ids] = batched_h
    in_w = sum(KCH * l for l in slot_lens)
    ins = []
    for c in range(N_CORES):
        a = np.zeros((128, in_w), dtype=np.float32)
        off = 0
        for s, L in enumerate(slot_lens):
            g = assign[s, c]
            n = int(lengths[g])
            # [n, D] -> [D, n] -> [KCH, 128, n] -> [128, KCH, n]
            xtg = padded[g, :n].T.reshape(KCH, 128, n).transpose(1, 0, 2)
            blk = a[:, off : off + KCH * L].reshape(128, KCH, L)
            blk[:, :, :n] = xtg
            off += KCH * L
        ins.append(a)
    return ins


def _host_gather(results, lengths, assign, slot_lens):
    full = np.zeros((B, MAXN, MAXN), dtype=np.float32)
    for c in range(N_CORES):
        o = results[c]["out"]
        off = 0
        for s, L in enumerate(slot_lens):
            g = assign[s, c]
            n = int(lengths[g])
            mb = _mb(L)
            blk = o[:, off : off + mb * L].reshape(128, mb, L)
            # rows 128*m + p -> blk[p, m, :]
            res = blk.transpose(1, 0, 2).reshape(mb * 128, L)
            full[g, :n, :n] = res[:n, :n]
            off += mb * L
    return full


def kernel(batched_h, graph_ids, pos_ids, B=None, max_n=None, **_ignored):
    from concourse.bass_utils import run_bass_kernel_spmd

    batched_h = np.asarray(batched_h, dtype=np.float32)
    graph_ids = np.asarray(graph_ids, dtype=np.int64)
    pos_ids = np.asarray(pos_ids, dtype=np.int64)

    lengths, assign, slot_lens = _plan(graph_ids)
    nc = _get_program(slot_lens)
    in_maps = [
        {"xt": a}
        for a in _host_prepare(batched_h, graph_ids, pos_ids, lengths, assign, slot_lens)
    ]
    res = run_bass_kernel_spmd(nc, in_maps, list(range(N_CORES)))
    return _host_gather(res.results, lengths, assign, slot_lens)



# revision 2
# speedup vs baseline: 1.2420x; 1.2420x over previous
"""Trainium2 Bass kernel for nn_InnerProductDecoder.

For each graph b: out[b] = P_b @ P_b^T where P_b is the zero-padded
[max_n, D] node-feature matrix of graph b (pad_sequence equivalent).

Strategy: data parallel over B (64 graphs / 8 cores). Graphs are
sorted by length and dealt round-robin so slot s on every core holds
a graph of length <= slot_len[s]; the SPMD program is built for the
slot-length profile, so each core only moves/computes its graphs'
ragged extents.

v2 over the f32r baseline:
  * fp16 end-to-end on device (input features, matmul operands, and
    stored output) — halves HBM traffic and runs the PE at full rate.
    PSUM accumulation stays fp32; measured rel err ~3e-4, well inside
    the 2e-2 gate. Host casts the fp16 result back to f32.
  * out[b] is symmetric, so only the upper-triangular 128-row blocks
    are computed/copied/stored (block m covers cols [128m, L)); the
    host mirrors the off-diagonal part. ~37% less PE work, copy work
    and output DMA.
  * PSUM->SBUF copies are split between DVE and ACT (the only two
    engines with PSUM read ports) to halve the copy-engine time.

Per-core raw-Bass pipeline:

  sync  (SP) : input DMA  xt slot block      -> SBUF x-buffer (x3)
  tensor(PE) : 4*mb fp16 matmuls             -> <=4 PSUM banks (x2 sets)
  vector(DVE): its share of PSUM -> SBUF fp16 copies (x3 out-buffers)
  scalar(ACT): its share of copies + output DMA out-buffer -> out

All sems are cleared at the tail (barrier + sem_clear) so re-executing
the loaded NEFF stays correct.
"""

import numpy as np

N_CORES = 8
B = 64
MAXN = 512
D = 512
PER_CORE = B // N_CORES  # 8 slots per core
KCH = D // 128  # 4 contraction chunks

_prog_cache = {}


def _mb(l):
    return (l + 127) // 128


def _slot_blocks(L):
    """Upper-triangular block list for one slot: (m, W, rows, t) where the
    copy of block m lands at out-buffer cols [t, t+W)."""
    bl, t = [], 0
    for m in range(_mb(L)):
        W = L - 128 * m
        rows = min(128, W)
        bl.append((m, W, rows, t))
        t += W
    return bl


def _build_program(slot_lens):
    import concourse.bass as bass
    from concourse import mybir

    f32 = mybir.dt.float32
    f16 = mybir.dt.float16
    nc = bass.Bass()

    J = len(slot_lens)
    lmax = max(slot_lens)
    blocks = [_slot_blocks(L) for L in slot_lens]
    T = [bl[-1][3] + bl[-1][1] for bl in blocks]  # out-buffer width per slot
    in_off = np.concatenate([[0], np.cumsum([KCH * l for l in slot_lens])])
    out_off = np.concatenate([[0], np.cumsum(T)])
    # m-blocks completed after slot j (prefix sums for sem values)
    cum_mb = np.concatenate([[0], np.cumsum([len(bl) for bl in blocks])])

    xt = nc.dram_tensor("xt", [128, int(in_off[-1])], f16, kind="ExternalInput")
    out = nc.dram_tensor("out", [128, int(out_off[-1])], f16, kind="ExternalOutput")

    # Split each slot's blocks between DVE and ACT, balancing estimated ns
    # (DVE ~1.04 ns/row + ~130 ns/instr, ACT ~0.83 ns/row + ~150 ns/instr).
    dve_blk, act_blk = [], []
    for bl in blocks:
        dc, ac, db, ab = 0.0, 0.0, [], []
        for blk in sorted(bl, key=lambda x: -x[1]):
            cd, ca = blk[1] * 1.04 + 130.0, blk[1] * 0.833 + 150.0
            if dc + cd <= ac + ca:
                db.append(blk)
                dc += cd
            else:
                ab.append(blk)
                ac += ca
        dve_blk.append(sorted(db))
        act_blk.append(sorted(ab))

    # Output DMA plan per slot: one full-height DMA for the 128-row blocks,
    # plus the partial last block with only its valid rows.
    out_plan = []
    for bl in blocks:
        m_last, W_last, rows_last, t_last = bl[-1]
        tot = t_last + W_last
        if rows_last == 128:
            out_plan.append([(128, 0, tot)])
        elif len(bl) == 1:
            out_plan.append([(rows_last, 0, tot)])
        else:
            out_plan.append([(128, 0, t_last), (rows_last, t_last, tot)])
    ndma = [len(p) for p in out_plan]
    # cumulative out-DMA count per buffer index i after each slot
    cum_out = {i: [0] for i in range(3)}
    for j in range(J):
        for i in range(3):
            cum_out[i].append(cum_out[i][-1] + (ndma[j] if j % 3 == i else 0))

    from contextlib import ExitStack

    with ExitStack() as st:
        xb = [
            st.enter_context(nc.sbuf_tensor(f"xb{i}", [128, KCH * lmax], f16))
            for i in range(3)
        ]
        obw = max(T)
        ob = [
            st.enter_context(nc.sbuf_tensor(f"ob{i}", [128, obw], f16))
            for i in range(3)
        ]
        ps = [
            st.enter_context(nc.psum_tensor(f"ps{i}", [128, 512], f32))
            for i in range(8)
        ]
        in_sems = [st.enter_context(nc.semaphore(f"in_sem{i}")) for i in range(3)]
        out_sems = [st.enter_context(nc.semaphore(f"out_sem{i}")) for i in range(3)]
        mm_sem = st.enter_context(nc.semaphore("mm_sem"))
        cp_sem = st.enter_context(nc.semaphore("cp_sem"))

        blk_ctx = nc.Block()
        block = blk_ctx.__enter__()

        @block.sync
        def _(sync):
            for j in range(J):
                if j >= 3:
                    # x-buffer j%3 free once slot j-3's last matmul ran
                    sync.wait_ge(mm_sem, int(cum_mb[j - 2]))
                w = KCH * slot_lens[j]
                src = xt[:, int(in_off[j]) : int(in_off[j]) + w]
                sync.dma_start(xb[j % 3][:, :w], src).then_inc(in_sems[j % 3], 16)
            for i in range(3):
                sync.wait_ge(out_sems[i], 16 * cum_out[i][J])

        @block.tensor
        def _(tensor):
            for j in range(J):
                L = slot_lens[j]
                tensor.wait_ge(in_sems[j % 3], 16 * (j // 3 + 1))
                if j >= 2:
                    # PSUM bank set j%2 free once slot j-2 fully copied out
                    tensor.wait_ge(cp_sem, int(cum_mb[j - 1]))
                pb = (j % 2) * 4
                for m, W, rows, _t in blocks[j]:
                    for k in range(KCH):
                        o = k * L + 128 * m
                        lhsT = xb[j % 3][:, o : o + rows]
                        rhs = xb[j % 3][:, o : k * L + L]
                        ins = nc.tensor.matmul(
                            ps[pb + m][:rows, :W], lhsT, rhs,
                            start=(k == 0), stop=(k == KCH - 1),
                        )
                        if k == KCH - 1:
                            ins.then_inc(mm_sem, 1)

        @block.vector
        def _(vector):
            for j in range(J):
                if not dve_blk[j]:
                    continue
                if j >= 3:
                    # out-buffer j%3 free once slot j-3's output DMAs landed
                    vector.wait_ge(out_sems[j % 3], 16 * cum_out[j % 3][j - 2])
                pb = (j % 2) * 4
                for m, W, rows, t in dve_blk[j]:
                    vector.wait_ge(mm_sem, int(cum_mb[j]) + m + 1)
                    nc.vector.tensor_copy(
                        ob[j % 3][:rows, t : t + W], ps[pb + m][:rows, :W]
                    ).then_inc(cp_sem, 1)

        @block.scalar
        def _(scalar):
            for j in range(J):
                if act_blk[j] and j >= 3:
                    scalar.wait_ge(out_sems[j % 3], 16 * cum_out[j % 3][j - 2])
                pb = (j % 2) * 4
                for m, W, rows, t in act_blk[j]:
                    scalar.wait_ge(mm_sem, int(cum_mb[j]) + m + 1)
                    nc.scalar.copy(
                        ob[j % 3][:rows, t : t + W], ps[pb + m][:rows, :W]
                    ).then_inc(cp_sem, 1)
                scalar.wait_ge(cp_sem, int(cum_mb[j + 1]))
                o0 = int(out_off[j])
                for rows, lo, hi in out_plan[j]:
                    scalar.dma_start(
                        out[:rows, o0 + lo : o0 + hi], ob[j % 3][:rows, lo:hi]
                    ).then_inc(out_sems[j % 3], 16)

        blk_ctx.__exit__(None, None, None)

        # Reset all sems to 0 so re-executing the loaded NEFF stays correct
        # (NRT does not clear sems between executes). Same sequence Tile uses.
        all_sems = in_sems + out_sems + [mm_sem, cp_sem]
        nc.all_engine_barrier()
        sem_nums = sorted(s.num for s in all_sems)
        lo, hi = sem_nums[0], sem_nums[-1] + 1
        assert sem_nums == list(range(lo, hi)), sem_nums
        nc.gpsimd.dma_reset(range(lo, hi))
        nc.gpsimd.sem_clear(range(lo, hi))
        nc.all_engine_barrier()

    return nc


def _plan(graph_ids):
    """Sort graphs by length desc, deal round-robin: slot s of core c gets
    rank 8*s + c. slot_lens[s] = max length within the slot = rank 8*s."""
    lengths = np.bincount(graph_ids, minlength=B)
    order = np.argsort(-lengths, kind="stable")  # rank -> graph id
    # round up to multiple of 32 so every fp16 matmul operand byte offset
    # (k*L*2) stays 64B-aligned (walrus ISA check on matmul operands)
    slot_lens = tuple(
        (int(lengths[order[N_CORES * s]]) + 31) // 32 * 32 for s in range(PER_CORE)
    )
    assign = order.reshape(PER_CORE, N_CORES)  # [slot, core] -> graph id
    return lengths, assign, slot_lens


def _get_program(slot_lens):
    if slot_lens not in _prog_cache:
        _prog_cache[slot_lens] = _build_program(list(slot_lens))
    return _prog_cache[slot_lens]


def _host_prepare(batched_h, graph_ids, pos_ids, lengths, assign, slot_lens):
    """Build per-core [128, sum(KCH*L_s)] fp16 inputs:
    xt[p, in_off[s] + k*L_s + n] = h[g_cs][node n][d=128k+p]."""
    padded = np.zeros((B, MAXN, D), dtype=np.float16)
    padded[graph_ids, pos_ids] = batched_h.astype(np.float16)
    in_w = sum(KCH * l for l in slot_lens)
    ins = []
    for c in range(N_CORES):
        a = np.zeros((128, in_w), dtype=np.float16)
        off = 0
        for s, L in enumerate(slot_lens):
            g = assign[s, c]
            n = int(lengths[g])
            # [n, D] -> [D, n] -> [KCH, 128, n] -> [128, KCH, n]
            xtg = padded[g, :n].T.reshape(KCH, 128, n).transpose(1, 0, 2)
            blk = a[:, off : off + KCH * L].reshape(128, KCH, L)
            blk[:, :, :n] = xtg
            off += KCH * L
        ins.append(a)
    return ins


def _host_gather(results, lengths, assign, slot_lens):
    """Paste upper-triangular blocks and mirror the strictly-lower part.
    Padded rows/cols beyond each graph's n hold exact zeros (zero-padded
    input rows), matching the reference output, so no cropping is needed."""
    full = np.zeros((B, MAXN, MAXN), dtype=np.float32)
    for c in range(N_CORES):
        o = results[c]["out"]
        off = 0
        for s, L in enumerate(slot_lens):
            g = assign[s, c]
            for m, W, rows, t in _slot_blocks(L):
                r0 = 128 * m
                blk = o[:rows, off + t : off + t + W].astype(np.float32)
                full[g, r0 : r0 + rows, r0 : r0 + W] = blk
                if W > rows:
                    full[g, r0 + rows : r0 + W, r0 : r0 + rows] = blk[:, rows:].T
            off += T_width(L)
    return full


def T_width(L):
    bl = _slot_blocks(L)
    return bl[-1][3] + bl[-1][1]


def kernel(batched_h, graph_ids, pos_ids, B=None, max_n=None, **_ignored):
    from concourse.bass_utils import run_bass_kernel_spmd

    batched_h = np.asarray(batched_h, dtype=np.float32)
    graph_ids = np.asarray(graph_ids, dtype=np.int64)
    pos_ids = np.asarray(pos_ids, dtype=np.int64)

    lengths, assign, slot_lens = _plan(graph_ids)
    nc = _get_program(slot_lens)
    in_maps = [
        {"xt": a}
        for a in _host_prepare(batched_h, graph_ids, pos_ids, lengths, assign, slot_lens)
    ]
    res = run_bass_kernel_spmd(nc, in_maps, list(range(N_CORES)))
    return _host_gather(res.results, lengths, assign, slot_lens)


# revision 6
# speedup vs baseline: 1.3042x; 1.0501x over previous
"""Trainium2 Bass kernel for nn_InnerProductDecoder.

For each graph b: out[b] = P_b @ P_b^T where P_b is the zero-padded
[max_n, D] node-feature matrix of graph b (pad_sequence equivalent).

Strategy: data parallel over B (64 graphs / 8 cores). Graphs are
sorted by length and dealt round-robin so slot s on every core holds
a graph of length <= slot_len[s]; the SPMD program is built for the
slot-length profile, so each core only moves/computes its graphs'
ragged extents.

v3 design notes:
  * fp16 end-to-end on device (PSUM accumulation fp32): halves HBM
    traffic vs f32 and runs the PE at full rate. Host casts back.
  * out[b] is symmetric: only upper-triangular 128-row blocks are
    computed/copied/stored (block m covers cols [128m, L)); host
    mirrors. ~37% less PE/copy/output-DMA work.
  * PSUM->SBUF copies split between DVE and ACT (the only engines
    with PSUM read ports), balanced including ACT's out-DMA issue
    cost; ACT's activation-table load is pre-triggered by a dummy
    SBUF copy so it doesn't land on the critical path.
  * PE warm-up: a burst of dummy matmuls at block start keeps the PE
    busy from t0 so the HAM throttle reaches K=8/8 before real work.
  * slot 0's input DMA is split in two k-chunk halves so the first
    real matmul starts ~1.5us earlier.
  * one merged output DMA per slot (a few KB of dead bytes beat an
    extra descriptor round-trip per slot).
  * tail: GpSimd alone waits for every semaphore's final value, then
    dma_reset + sem_clear (re-execution safety); other engines flow
    straight into the Block-exit barrier.

Per-core raw-Bass pipeline:

  sync  (SP) : input DMA  xt slot block      -> SBUF x-buffer (x3)
  tensor(PE) : 4*mb fp16 matmuls             -> <=4 PSUM banks (x2 sets)
  vector(DVE): its share of PSUM -> SBUF fp16 copies (x3 out-buffers)
  scalar(ACT): its share of copies + output DMA out-buffer -> out
  gpsimd     : tail sem/dma reset
"""

import numpy as np

N_CORES = 8
B = 64
MAXN = 512
D = 512
PER_CORE = B // N_CORES  # 8 slots per core
KCH = D // 128  # 4 contraction chunks
NWARM = 6  # PE warm-up dummy matmuls (W=256 each, ~2.5us)

_prog_cache = {}


def _mb(l):
    return (l + 127) // 128


def _slot_blocks(L):
    """Upper-triangular block list for one slot: (m, W, rows, t) where the
    copy of block m lands at out-buffer cols [t, t+W)."""
    bl, t = [], 0
    for m in range(_mb(L)):
        W = L - 128 * m
        rows = min(128, W)
        bl.append((m, W, rows, t))
        t += W
    return bl


def _build_program(slot_lens):
    import concourse.bass as bass
    from concourse import mybir

    f32 = mybir.dt.float32
    f16 = mybir.dt.float16
    nc = bass.Bass()

    J = len(slot_lens)
    lmax = max(slot_lens)
    blocks = [_slot_blocks(L) for L in slot_lens]
    T = [bl[-1][3] + bl[-1][1] for bl in blocks]  # out width per slot
    in_off = np.concatenate([[0], np.cumsum([KCH * l for l in slot_lens])])
    out_off = np.concatenate([[0], np.cumsum(T)])
    # m-blocks completed after slot j (prefix sums for sem values)
    cum_mb = np.concatenate([[0], np.cumsum([len(bl) for bl in blocks])])

    # input DMA chunking: slot 0 arrives in two k-halves so the PE can
    # start as soon as half the slot is resident
    in_chunks = [[(0, 2), (2, 4)]] + [[(0, KCH)]] * (J - 1)
    in_base = []  # per-slot: completed input DMAs on its buffer before it
    _cnt = [0, 0, 0]
    for j in range(J):
        in_base.append(_cnt[j % 3])
        _cnt[j % 3] += len(in_chunks[j])
    n_in = list(_cnt)

    xt = nc.dram_tensor("xt", [128, int(in_off[-1])], f16, kind="ExternalInput")
    out = nc.dram_tensor("out", [128, int(out_off[-1])], f16, kind="ExternalOutput")

    # Split each slot's blocks between DVE and ACT, balancing estimated ns
    # (DVE ~1.04 ns/row + ~130 ns/instr; ACT ~0.83 ns/row + ~150 ns/instr
    # plus ~650 ns for the slot's out-DMA issue).
    dve_blk, act_blk = [], []
    for bl in blocks:
        dc, ac, db, ab = 0.0, 650.0, [], []
        for blk in sorted(bl, key=lambda x: -x[1]):
            cd, ca = blk[1] * 1.04 + 130.0, blk[1] * 0.833 + 150.0
            if dc + cd <= ac + ca:
                db.append(blk)
                dc += cd
            else:
                ab.append(blk)
                ac += ca
        dve_blk.append(sorted(db))
        act_blk.append(sorted(ab))

    # one merged output DMA per slot (dead rows of the partial last block
    # are written too and ignored by the host)
    cum_out = {i: [0] for i in range(3)}
    for j in range(J):
        for i in range(3):
            cum_out[i].append(cum_out[i][-1] + (1 if j % 3 == i else 0))

    from contextlib import ExitStack

    with ExitStack() as st:
        xb = [
            st.enter_context(nc.sbuf_tensor(f"xb{i}", [128, KCH * lmax], f16))
            for i in range(3)
        ]
        wb = st.enter_context(nc.sbuf_tensor("wb", [128, 256], f16))
        obw = max(T)
        ob = [
            st.enter_context(nc.sbuf_tensor(f"ob{i}", [128, obw], f16))
            for i in range(3)
        ]
        ps = [
            st.enter_context(nc.psum_tensor(f"ps{i}", [128, 512], f32))
            for i in range(8)
        ]
        in_sems = [st.enter_context(nc.semaphore(f"in_sem{i}")) for i in range(3)]
        out_sems = [st.enter_context(nc.semaphore(f"out_sem{i}")) for i in range(3)]
        mm_sem = st.enter_context(nc.semaphore("mm_sem"))
        cp_sem = st.enter_context(nc.semaphore("cp_sem"))
        all_sems = in_sems + out_sems + [mm_sem, cp_sem]
        sem_nums = sorted(s.num for s in all_sems)
        lo, hi = sem_nums[0], sem_nums[-1] + 1
        assert sem_nums == list(range(lo, hi)), sem_nums

        blk_ctx = nc.Block()
        block = blk_ctx.__enter__()

        @block.sync
        def _(sync):
            for j in range(J):
                if j >= 3:
                    # x-buffer j%3 free once slot j-3's last matmul ran
                    sync.wait_ge(mm_sem, int(cum_mb[j - 2]))
                L = slot_lens[j]
                for klo, khi in in_chunks[j]:
                    src = xt[:, int(in_off[j]) + klo * L : int(in_off[j]) + khi * L]
                    sync.dma_start(
                        xb[j % 3][:, klo * L : khi * L], src
                    ).then_inc(in_sems[j % 3], 16)

        @block.tensor
        def _(tensor):
            # HAM warm-up: garbage matmuls keep the PE busy while the first
            # input DMA is in flight, so K=8/8 engages before real work.
            for w in range(NWARM):
                nc.tensor.matmul(
                    ps[w % 4][:128, :256], wb[:, :128], wb[:, :256],
                    start=True, stop=True, skip_group_check=True,
                )
            for j in range(J):
                L = slot_lens[j]
                if j >= 2:
                    # PSUM bank set j%2 free once slot j-2 fully copied out
                    tensor.wait_ge(cp_sem, int(cum_mb[j - 1]))
                pb = (j % 2) * 4
                for ci, (klo, khi) in enumerate(in_chunks[j]):
                    tensor.wait_ge(in_sems[j % 3], 16 * (in_base[j] + ci + 1))
                    for m, W, rows, _t in blocks[j]:
                        for k in range(klo, khi):
                            o = k * L + 128 * m
                            lhsT = xb[j % 3][:, o : o + rows]
                            rhs = xb[j % 3][:, o : k * L + L]
                            ins = nc.tensor.matmul(
                                ps[pb + m][:rows, :W], lhsT, rhs,
                                start=(k == 0), stop=(k == KCH - 1),
                                skip_group_check=(len(in_chunks[j]) > 1),
                            )
                            if k == KCH - 1:
                                ins.then_inc(mm_sem, 1)

        @block.vector
        def _(vector):
            for j in range(J):
                if not dve_blk[j]:
                    continue
                if j >= 3:
                    # out-buffer j%3 free once slot j-3's output DMA landed
                    vector.wait_ge(out_sems[j % 3], 16 * cum_out[j % 3][j - 2])
                pb = (j % 2) * 4
                for m, W, rows, t in dve_blk[j]:
                    vector.wait_ge(mm_sem, int(cum_mb[j]) + m + 1)
                    nc.vector.tensor_copy(
                        ob[j % 3][:rows, t : t + W], ps[pb + m][:rows, :W]
                    ).then_inc(cp_sem, 1)

        @block.scalar
        def _(scalar):
            # pre-trigger the ACT table load for Copy off the critical path
            # (SBUF->SBUF dummy; slot 0's real copies overwrite this region)
            nc.scalar.copy(ob[0][:1, :8], wb[:1, :8])
            for j in range(J):
                if act_blk[j] and j >= 3:
                    scalar.wait_ge(out_sems[j % 3], 16 * cum_out[j % 3][j - 2])
                pb = (j % 2) * 4
                for m, W, rows, t in act_blk[j]:
                    scalar.wait_ge(mm_sem, int(cum_mb[j]) + m + 1)
                    nc.scalar.copy(
                        ob[j % 3][:rows, t : t + W], ps[pb + m][:rows, :W]
                    ).then_inc(cp_sem, 1)
                scalar.wait_ge(cp_sem, int(cum_mb[j + 1]))
                o0 = int(out_off[j])
                scalar.dma_start(
                    out[:, o0 : o0 + T[j]], ob[j % 3][:, : T[j]]
                ).then_inc(out_sems[j % 3], 16)

        @block.gpsimd
        def _(g):
            # Tail: wait for every semaphore's final value, then reset them
            # so re-executing the loaded NEFF stays correct (NRT does not
            # clear sems between executes). Runs concurrently with the other
            # engines' Block-exit barrier entry.
            for i in range(3):
                g.wait_ge(in_sems[i], 16 * n_in[i])
                g.wait_ge(out_sems[i], 16 * cum_out[i][J])
            g.wait_ge(mm_sem, int(cum_mb[J]))
            g.wait_ge(cp_sem, int(cum_mb[J]))
            g.dma_reset(range(lo, hi))
            g.sem_clear(range(lo, hi))

        blk_ctx.__exit__(None, None, None)

    return nc


def _plan(graph_ids):
    """Sort graphs by length desc, deal round-robin: slot s of core c gets
    rank 8*s + c. slot_lens[s] = max length within the slot = rank 8*s."""
    lengths = np.bincount(graph_ids, minlength=B)
    order = np.argsort(-lengths, kind="stable")  # rank -> graph id
    # round up to multiple of 32 so every fp16 matmul operand byte offset
    # (k*L*2) stays 64B-aligned (walrus ISA check on matmul operands)
    slot_lens = tuple(
        (int(lengths[order[N_CORES * s]]) + 31) // 32 * 32 for s in range(PER_CORE)
    )
    assign = order.reshape(PER_CORE, N_CORES)  # [slot, core] -> graph id
    return lengths, assign, slot_lens


def _get_program(slot_lens):
    if slot_lens not in _prog_cache:
        _prog_cache[slot_lens] = _build_program(list(slot_lens))
    return _prog_cache[slot_lens]


def _host_prepare(batched_h, graph_ids, pos_ids, lengths, assign, slot_lens):
    """Build per-core [128, sum(KCH*L_s)] fp16 inputs:
    xt[p, in_off[s] + k*L_s + n] = h[g_cs][node n][d=128k+p]."""
    padded = np.zeros((B, MAXN, D), dtype=np.float16)
    padded[graph_ids, pos_ids] = batched_h.astype(np.float16)
    in_w = sum(KCH * l for l in slot_lens)
    ins = []
    for c in range(N_CORES):
        a = np.zeros((128, in_w), dtype=np.float16)
        off = 0
        for s, L in enumerate(slot_lens):
            g = assign[s, c]
            n = int(lengths[g])
            # [n, D] -> [D, n] -> [KCH, 128, n] -> [128, KCH, n]
            xtg = padded[g, :n].T.reshape(KCH, 128, n).transpose(1, 0, 2)
            blk = a[:, off : off + KCH * L].reshape(128, KCH, L)
            blk[:, :, :n] = xtg
            off += KCH * L
        ins.append(a)
    return ins


def _host_gather(results, lengths, assign, slot_lens):
    """Paste upper-triangular blocks and mirror the strictly-lower part.
    Padded rows/cols beyond each graph's n hold exact zeros (zero-padded
    input rows), matching the reference output, so no cropping is needed."""
    full = np.zeros((B, MAXN, MAXN), dtype=np.float32)
    for c in range(N_CORES):
        o = results[c]["out"]
        off = 0
        for s, L in enumerate(slot_lens):
            g = assign[s, c]
            bl = _slot_blocks(L)
            for m, W, rows, t in bl:
                r0 = 128 * m
                blk = o[:rows, off + t : off + t + W].astype(np.float32)
                full[g, r0 : r0 + rows, r0 : r0 + W] = blk
                if W > rows:
                    full[g, r0 + rows : r0 + W, r0 : r0 + rows] = blk[:, rows:].T
            off += bl[-1][3] + bl[-1][1]
    return full


def kernel(batched_h, graph_ids, pos_ids, B=None, max_n=None, **_ignored):
    from concourse.bass_utils import run_bass_kernel_spmd

    batched_h = np.asarray(batched_h, dtype=np.float32)
    graph_ids = np.asarray(graph_ids, dtype=np.int64)
    pos_ids = np.asarray(pos_ids, dtype=np.int64)

    lengths, assign, slot_lens = _plan(graph_ids)
    nc = _get_program(slot_lens)
    in_maps = [
        {"xt": a}
        for a in _host_prepare(batched_h, graph_ids, pos_ids, lengths, assign, slot_lens)
    ]
    res = run_bass_kernel_spmd(nc, in_maps, list(range(N_CORES)))
    return _host_gather(res.results, lengths, assign, slot_lens)
